# revision 14
# baseline (speedup 1.0000x reference)
"""Trainium2 Bass kernel for ConvOffset: Conv2D(3x3, fixed one-hot-tap kernel) + Dense.

The staged conv kernel is zero everywhere except the center tap [1,1], which is
all-ones over (cin, cout).  Folding the conv kernel into the Dense weight W:

    out[b,h,w,o] = sum_i x[b,h,w,i] * M11[i,o] + bias @ W,
    M11[i,o]     = sum_c K[1,1,i,c] * W[c,o]

and because K[1,1] has identical rows (all-ones), M11 is rank-1 with identical
rows m = K[1,1][0] @ W, so

    out[b,h,w,o] = (sum_i x[b,h,w,i]) * m[o]

i.e. a channel-sum reduction followed by a rank-1 outer-product broadcast.
This is verified on the host at runtime; if the structure doesn't hold, an
exact (slow) numpy conv fallback is used instead.

Device kernel (per NeuronCore, data-parallel over the batch: 1 image/core):
  - fp16 in DRAM for both streams: the kernel is pure DMA-bound (the fp32
    version sat at 97% of the 358 GB/s per-core HBM roofline), so halving
    bytes/element halves exec time.  fp16 keeps 10 mantissa bits; measured
    end-to-end rel err ~6e-4, far inside the 2e-2 gate.  All arithmetic on
    device stays fp32 (DVE reduce accumulates in fp32).
  - tile = 128 partitions x (R=64 positions x 128 channels) fp16,
    partition-contiguous position mapping so every DMA reads/writes
    R*256B = 16KB contiguous per partition
  - measured DVE rates (microbench.py): 1x = 1.04ns/elem/partition for any
    op with an fp32 or stride-0 operand; all-packed-fp16 tensor_tensor runs
    the 2x mode (0.56ns/elem); tensor_reduce is always 1x; ScalarE ACTIVATE
    is 0.9ns/elem and accepts stride-0 broadcast inputs.  So:
      fold1/fold2 (DVE, 2x): pairwise channel adds 128->64->32 packed fp16
      reduce (DVE, 1x): 32-wide tensor_reduce -> S[p, r] fp16
      bcast (ScalarE):  S -> S_rep[p, r, c] fp16 (stride-0 in, full rate)
      mult (DVE, 2x):   S_rep * wt_rep, all packed fp16 -> out tile
    which keeps VectorE (~88us) and ScalarE (~59us) under the ~94us DMA
    roofline instead of 140us of 1x VectorE work.
  - DMA out
"""

import sys

import numpy as np

for _p in ("/opt/trn_rl_repo", "/root/.axon_site/_ro/trn_rl_repo"):
    if _p not in sys.path:
        sys.path.insert(0, _p)

P = 128           # SBUF partitions
C = 128           # channels (cin == cout)
R = 64            # positions per partition per tile (16KB fp16 spans)
T = 8             # tiles per core;  P * R * T == 256 * 256 positions
NPOS = P * R * T  # 65536 positions per core (one 256x256 image)
N_CORES = 8

_NC_CACHE = {}


def _build_nc():
    import concourse.bass as bass
    import concourse.bacc as bacc
    import concourse.tile as tile
    from concourse import mybir

    nc = bacc.Bacc(None)
    x = nc.dram_tensor("x", [NPOS, C], mybir.dt.float16, kind="ExternalInput")
    w = nc.dram_tensor("wsum", [P, (R // 2) * C], mybir.dt.float16, kind="ExternalInput")
    out = nc.dram_tensor("out", [NPOS, C], mybir.dt.float16, kind="ExternalOutput")

    # position = ((t*P + p)*R + r): per (t, p) the (r, c) block is one
    # contiguous R*512B span in DRAM -> line-rate DMA descriptors.
    xr = x[:].rearrange("(t p r) c -> t p r c", p=P, r=R)
    outr = out[:].rearrange("(t p r) c -> t p r c", p=P, r=R)

    H = R // 2
    with tile.TileContext(nc) as tc:
        with (
            tc.tile_pool(name="xin", bufs=4) as xin_pool,
            tc.tile_pool(name="oout", bufs=4) as out_pool,
            tc.tile_pool(name="f1", bufs=4) as f1_pool,
            tc.tile_pool(name="f2", bufs=4) as f2_pool,
            tc.tile_pool(name="s", bufs=8) as s_pool,
            tc.tile_pool(name="srep", bufs=4) as srep_pool,
            tc.tile_pool(name="const", bufs=1) as const_pool,
        ):
            # Load the (replicated) weight row via the GpSimd (SWDGE) ring so
            # the SP ring starts streaming x tiles immediately.  One [P, H, C]
            # replication serves every half-tile's mult.
            wt = const_pool.tile([P, H, C], mybir.dt.float16)
            nc.gpsimd.dma_start(
                out=wt[:], in_=w[:].rearrange("p (r c) -> p r c", r=H)
            )

            # Loads and stores share the SP ring on purpose: the scheduler
            # batches same-direction DMAs, and phase-separated R/W streams
            # keep each SDMA engine at line rate (strict in/out alternation
            # measured ~20% slower from HBM direction turnarounds).
            # Loads on the Sync HW queue, stores on the Scalar HW queue: two
            # independent DGE dispatchers keep the 16 shared SDMA engines
            # fed (one queue measured only ~75% engine occupancy).
            def chunk(xt, ot, t, lo, hi):
                hc = hi - lo
                f1 = f1_pool.tile([P, hc, C // 2], mybir.dt.float16)
                nc.vector.tensor_add(
                    out=f1[:],
                    in0=xt[:, lo:hi, 0 : C // 2],
                    in1=xt[:, lo:hi, C // 2 : C],
                )
                f2 = f2_pool.tile([P, hc, C // 4], mybir.dt.float16)
                nc.vector.tensor_add(
                    out=f2[:],
                    in0=f1[:, :, 0 : C // 4],
                    in1=f1[:, :, C // 4 : C // 2],
                )
                s = s_pool.tile([P, hc], mybir.dt.float16)
                with nc.allow_low_precision(
                    reason="fp16 channel-sum; 2e-2 gate, ~2^-11 rounding"
                ):
                    nc.vector.tensor_reduce(
                        out=s[:],
                        in_=f2[:],
                        axis=mybir.AxisListType.X,
                        op=mybir.AluOpType.add,
                    )
                srep = srep_pool.tile([P, hc, C], mybir.dt.float16)
                nc.scalar.activation(
                    out=srep[:],
                    in_=s[:].to_broadcast((P, hc, C)),
                    func=mybir.ActivationFunctionType.Copy,
                )
                nc.vector.tensor_mul(
                    out=ot[:, lo:hi, :], in0=srep[:], in1=wt[:, 0:hc, :]
                )
                nc.scalar.dma_start(out=outr[t][:, lo:hi, :], in_=ot[:, lo:hi, :])

            for t in range(T):
                xt = xin_pool.tile([P, R, C], mybir.dt.float16)
                nc.sync.dma_start(out=xt[:], in_=xr[t])

                ot = out_pool.tile([P, R, C], mybir.dt.float16)
                # Quarter-chunks on the first/last tile shorten the pipeline
                # head (first store ready sooner) and tail (last store's
                # compute chain is half as deep); half-chunks elsewhere.
                n = 4 if t in (0, T - 1) else 2
                hc = R // n
                for h in range(n):
                    chunk(xt, ot, t, h * hc, (h + 1) * hc)

    nc.finalize()
    return nc


def _get_nc():
    if "nc" not in _NC_CACHE:
        _NC_CACHE["nc"] = _build_nc()
    return _NC_CACHE["nc"]


def _fallback_numpy(X, K, b, Wd):
    """Exact general path: full 3x3 SAME conv + bias, then Dense. Only used if
    the staged inputs ever stop matching the one-hot-tap structure."""
    B, H, Wi, Ci = X.shape
    Co = Wd.shape[1]
    M = np.einsum("xyic,co->xyio", K, Wd).astype(np.float32)
    Xp = np.zeros((B, H + 2, Wi + 2, Ci), np.float32)
    Xp[:, 1:-1, 1:-1, :] = X
    out = np.zeros((B, H, Wi, Co), np.float32)
    for dx in range(3):
        for dy in range(3):
            out += Xp[:, dx : dx + H, dy : dy + Wi, :] @ M[dx, dy]
    out += b @ Wd
    return out.astype(np.float32)


def _install_ntff_hook():
    """Provide antenv.axon_hooks if the image lacks it (slim ctypes NTFF hook,
    same mechanism as trn_agent_boot.trn_boot._ntff_profile_via_ctypes)."""
    try:
        from antenv.axon_hooks import get_axon_ntff_profile_hook  # noqa: F401

        return
    except ImportError:
        pass

    import contextlib
    import ctypes
    import types

    so_path = "/opt/axon/libaxon_pjrt.so"
    lib = ctypes.CDLL(so_path)
    if not hasattr(lib, "axon_start_nrt_profile"):
        hook = None
    else:
        lib.axon_start_nrt_profile.argtypes = [
            ctypes.POINTER(ctypes.c_int64),
            ctypes.c_size_t,
        ]
        lib.axon_start_nrt_profile.restype = ctypes.c_int64
        lib.axon_stop_nrt_profile.argtypes = [ctypes.c_char_p]
        lib.axon_stop_nrt_profile.restype = ctypes.c_int64

        @contextlib.contextmanager
        def hook(output_dir, device_ids):
            import jax

            jax.devices()
            if device_ids:
                ids = (ctypes.c_int64 * len(device_ids))(*device_ids)
                rc = lib.axon_start_nrt_profile(ids, len(device_ids))
            else:
                rc = lib.axon_start_nrt_profile(None, 0)
            if rc != 0:
                raise RuntimeError(f"axon_start_nrt_profile rc={rc}")
            try:
                yield
            finally:
                n = lib.axon_stop_nrt_profile(str(output_dir).encode())
                print(f"ntff profile: {n} file(s) written to {output_dir}")

    mod = types.ModuleType("antenv.axon_hooks")
    mod.get_axon_ntff_profile_hook = lambda: hook
    mod.set_axon_ntff_profile_hook = lambda h: None
    sys.modules["antenv.axon_hooks"] = mod
    import antenv

    antenv.axon_hooks = mod


def _run_device(in_maps, trace=False, **kwargs):
    import concourse.bass_utils as bu

    if trace:
        _install_ntff_hook()
        # Zero-egress container: keep artifacts local instead of uploading.
        bu.upload_artifacts = lambda tmpdir: str(tmpdir)

    nc = _get_nc()
    return bu.run_bass_kernel_spmd(
        nc, in_maps, list(range(N_CORES)), trace=trace, **kwargs
    )


def _prepare(inputs, kernel, bias, W):
    X = np.ascontiguousarray(np.asarray(inputs, dtype=np.float32))
    K = np.asarray(kernel, dtype=np.float32)
    b = np.asarray(bias, dtype=np.float32)
    Wd = np.asarray(W, dtype=np.float32)

    structure_ok = (
        X.shape == (N_CORES, 256, 256, C)
        and K.shape == (3, 3, C, C)
        and Wd.shape == (C, C)
        and all(
            not np.any(K[dx, dy])
            for dx in range(3)
            for dy in range(3)
            if (dx, dy) != (1, 1)
        )
        and bool(np.all(K[1, 1] == K[1, 1][0:1, :]))
    )
    if not structure_ok:
        return None

    m = (K[1, 1][0:1, :] @ Wd)[0]          # (C,) folded rank-1 weight
    b_eff = (b @ Wd).astype(np.float32)    # (C,) folded bias (zeros in practice)
    wsum_rep = np.ascontiguousarray(
        np.broadcast_to(m.astype(np.float16), (P, R // 2, C)).reshape(P, (R // 2) * C)
    )
    Xf = X.astype(np.float16).reshape(N_CORES, NPOS, C)
    in_maps = [{"x": Xf[i], "wsum": wsum_rep} for i in range(N_CORES)]
    return in_maps, b_eff


def kernel(inputs, kernel, bias, W):
    prep = _prepare(inputs, kernel, bias, W)
    if prep is None:
        return _fallback_numpy(
            np.asarray(inputs, np.float32),
            np.asarray(kernel, np.float32),
            np.asarray(bias, np.float32),
            np.asarray(W, np.float32),
        )
    in_maps, b_eff = prep

    try:
        res = _run_device(in_maps, trace=False)
    except Exception:
        return _fallback_numpy(
            np.asarray(inputs, np.float32),
            np.asarray(kernel, np.float32),
            np.asarray(bias, np.float32),
            np.asarray(W, np.float32),
        )
    out = np.stack([res.results[i]["out"] for i in range(N_CORES)])
    out = out.reshape(N_CORES, 256, 256, C).astype(np.float32)
    if np.any(b_eff):
        out = (out + b_eff).astype(np.float32)
    return out


def kernel_traced(inputs, kernel, bias, W, **kwargs):
    """Like kernel(), but profiles on HW; returns (output, BassKernelResults)."""
    prep = _prepare(inputs, kernel, bias, W)
    assert prep is not None, "inputs do not match the staged structure"
    in_maps, b_eff = prep
    res = _run_device(in_maps, trace=True, **kwargs)
    out = np.stack([res.results[i]["out"] for i in range(N_CORES)])
    out = out.reshape(N_CORES, 256, 256, C).astype(np.float32)
    if np.any(b_eff):
        out = (out + b_eff).astype(np.float32)
    return out, res

